# revision 3
# baseline (speedup 1.0000x reference)
"""Conditional InstanceNorm2d on 8 Trainium2 NeuronCores (Bass/Tile).

Reference semantics (torch InstanceNorm2d, affine=True, biased var):
    out[b,c,h,w] = (x[b,c,h,w] - mean[b,c]) * rsqrt(var[b,c] + 1e-5)
                   * gamma[style_id[b], c] + beta[style_id[b], c]

Sharding: data-parallel along batch. Each of the 8 cores gets 4 samples,
viewed as [1024 (b,c) rows, 4096 spatial] f32. Per 128-row tile:
  - DMA load [128, 4096] (HWDGE, 2 MiB, each partition line 16 KiB contiguous)
  - Vector engine: 8x bn_stats(512) + bn_aggr -> per-row (mean, var)
  - rstd = 1/sqrt(var+eps) (ACT sqrt + DVE reciprocal)
  - s = gamma_row * rstd ; t = beta_row - mean * s   (tiny [128,1] DVE ops)
  - Scalar engine: out = x * s + t (one fused ACT pass, in-place)
  - DMA store (SWDGE)
The [16,256] gamma/beta tables are gathered by style_id on host (32 lookups)
as part of input sharding; each core receives its per-row scale/shift.
"""

import sys

_REPO = "/opt/trn_rl_repo"
if _REPO not in sys.path:
    sys.path.insert(0, _REPO)

import numpy as np

import concourse.bacc as bacc
import concourse.bass as bass
import concourse.tile as tile
from concourse import mybir
from concourse.bass_utils import run_bass_kernel_spmd

B, C, H, W = 32, 256, 64, 64
S = 16
N_CORES = 8
B_PER = B // N_CORES  # 4 samples per core
ROWS = B_PER * C  # 1024 (b,c) rows per core
D = H * W  # 4096 spatial elements per row
P = 128  # SBUF partitions
NT = ROWS // P  # 8 row-tiles per core
CHUNK = 512  # bn_stats hardware max free size
NCHUNK = D // CHUNK  # 8 bn_stats calls per tile
EPS = 1e-5
F32 = mybir.dt.float32

X_BUFS = 3  # row-tile pool depth (triple buffer: load/compute/store)

_NC_CACHE = {}


def _build(n_reps=1):
    """Build the per-core kernel. n_reps>1 wraps the body in an in-NEFF
    For_i loop (identical idempotent work) for device-side timing via
    (T(n_reps) - T(1)) / (n_reps - 1)."""
    if n_reps in _NC_CACHE:
        return _NC_CACHE[n_reps]

    nc = bacc.Bacc(
        "TRN2",
        target_bir_lowering=False,
        debug=False,
        enable_asserts=False,
        num_devices=N_CORES,
    )
    x = nc.dram_tensor("x", [ROWS, D], F32, kind="ExternalInput").ap()
    g = nc.dram_tensor("g", [P, NT], F32, kind="ExternalInput").ap()
    bt = nc.dram_tensor("bt", [P, NT], F32, kind="ExternalInput").ap()
    out = nc.dram_tensor("out", [ROWS, D], F32, kind="ExternalOutput").ap()

    xr = x.rearrange("(n p) d -> n p d", p=P)
    outr = out.rearrange("(n p) d -> n p d", p=P)

    with tile.TileContext(nc) as tc:
        with (
            tc.tile_pool(name="xp", bufs=X_BUFS) as xp,
            tc.tile_pool(name="sp", bufs=3) as sp,
            tc.tile_pool(name="ones", bufs=1) as ones,
        ):
            g_sb = ones.tile([P, NT], F32, tag="g")
            b_sb = ones.tile([P, NT], F32, tag="b")
            eps_sb = ones.tile([P, 1], F32, tag="eps")
            nc.gpsimd.dma_start(out=g_sb[:], in_=g)
            nc.gpsimd.dma_start(out=b_sb[:], in_=bt)
            nc.vector.memset(eps_sb[:], EPS)

            def body():
                for t in range(NT):
                    xt = xp.tile([P, D], F32, tag="x")
                    nc.sync.dma_start(out=xt[:], in_=xr[t])

                    stats = sp.tile([P, NCHUNK, 6], F32, tag="stats")
                    for c in range(NCHUNK):
                        nc.vector.bn_stats(
                            out=stats[:, c, :], in_=xt[:, bass.ts(c, CHUNK)]
                        )
                    mv = sp.tile([P, 2], F32, tag="mv")
                    nc.vector.bn_aggr(out=mv[:], in_=stats[:])

                    # rstd = 1 / sqrt(var + eps)
                    rstd = sp.tile([P, 1], F32, tag="rstd")
                    nc.scalar.activation(
                        out=rstd[:],
                        in_=mv[:, 1:2],
                        func=mybir.ActivationFunctionType.Sqrt,
                        bias=eps_sb[:],
                        scale=1.0,
                    )
                    nc.vector.reciprocal(out=rstd[:], in_=rstd[:])

                    # s = gamma * rstd ; t = beta - mean * s
                    s_t = sp.tile([P, 1], F32, tag="s")
                    nc.vector.tensor_mul(s_t[:], g_sb[:, t : t + 1], rstd[:])
                    tt = sp.tile([P, 1], F32, tag="t")
                    nc.vector.tensor_mul(tt[:], mv[:, 0:1], s_t[:])
                    nc.vector.tensor_sub(tt[:], b_sb[:, t : t + 1], tt[:])

                    # out = x * s + t, fused on scalar (ACT) engine, in place
                    nc.scalar.activation(
                        out=xt[:],
                        in_=xt[:],
                        func=mybir.ActivationFunctionType.Identity,
                        bias=tt[:],
                        scale=s_t[:],
                    )
                    nc.gpsimd.dma_start(out=outr[t], in_=xt[:])

            if n_reps == 1:
                body()
            else:
                with tc.For_i(0, n_reps, 1):
                    body()

    nc.compile()
    _NC_CACHE[n_reps] = nc
    return nc


def make_in_maps(x, style_id, gamma, beta):
    """Host-side sharding: batch-split x, style-gather + split gamma/beta."""
    x = np.asarray(x, dtype=np.float32)
    style_id = np.asarray(style_id).astype(np.int64)
    gamma = np.asarray(gamma, dtype=np.float32)
    beta = np.asarray(beta, dtype=np.float32)
    g_all = gamma[style_id]  # [B, C]
    b_all = beta[style_id]  # [B, C]
    in_maps = []
    for i in range(N_CORES):
        sl = slice(i * B_PER, (i + 1) * B_PER)
        xs = np.ascontiguousarray(x[sl]).reshape(ROWS, D)
        # row r = b*C + c ; tile t covers rows t*128..t*128+127; SBUF wants
        # [p, t] layout so g_sb[p, t] = g_flat[t*128 + p]
        gs = np.ascontiguousarray(g_all[sl].reshape(NT, P).T)
        bs = np.ascontiguousarray(b_all[sl].reshape(NT, P).T)
        in_maps.append({"x": xs, "g": gs, "bt": bs})
    return in_maps


def run_sharded(in_maps, **kwargs):
    """Run the SPMD kernel; kwargs forwarded to run_bass_kernel_spmd."""
    nc = _build()
    return run_bass_kernel_spmd(nc, in_maps, list(range(N_CORES)), **kwargs)


def kernel(**inputs):
    in_maps = make_in_maps(
        inputs["x"], inputs["style_id"], inputs["gamma"], inputs["beta"]
    )
    res = run_sharded(in_maps)
    out = np.empty((B, C, H, W), dtype=np.float32)
    for i in range(N_CORES):
        out[i * B_PER : (i + 1) * B_PER] = np.asarray(
            res.results[i]["out"]
        ).reshape(B_PER, C, H, W)
    return out
